# revision 29
# baseline (speedup 1.0000x reference)
"""TRN2 Bass kernel for nn_MoEPositionwiseFFN: kernel(**inputs) -> np.ndarray.

Self-contained: builds (and caches) an 8-core SPMD Bass/Tile NEFF
implementing the MoE positionwise FFN (router + top-2 + capacity drop +
expert FFN + gated combine), shards the full inputs across the 8
NeuronCores (expert-parallel weights, data-parallel tokens, replicated
bf16 x for local dispatch gathers), runs it, and reassembles the full
[B, T, D] output.

Layout note: dispatch/combine slots use a chunk-major flat coordinate
  off2 = (slot//CCHUNK)*(NC*CCHUNK) + e*CCHUNK + slot%CCHUNK
so that the per-FFN-chunk AllGather output slices are contiguous in
all_out, letting the collective overlap the remaining FFN chunks.
"""

import sys

for _p in ("/opt/trn_rl_repo", "/opt/pypackages"):
    if _p not in sys.path:
        sys.path.insert(0, _p)


from dataclasses import dataclass

import numpy as np

import concourse.bass as bass
import concourse.bacc as bacc
import concourse.tile as tile
import concourse.mybir as mybir

FP32 = mybir.dt.float32
BF16 = mybir.dt.bfloat16
I32 = mybir.dt.int32
I16 = mybir.dt.int16
U16 = mybir.dt.uint16
AF = mybir.ActivationFunctionType
ALU = mybir.AluOpType
AX = mybir.AxisListType


@dataclass
class Cfg:
    ncores: int = 8
    E: int = 8
    K: int = 2
    D: int = 1024
    H: int = 4096
    TPC: int = 2048          # tokens per core
    cap: int = 2458          # reference capacity
    CAPP: int = 2560         # padded capacity (multiple of CCHUNK, > cap)
    CCHUNK: int = 512        # FFN token chunk (multiple of 128, <= 512)
    NBIS: int = 30           # bisection iterations (covers bits 0..0x3F800000)

    @property
    def N(self):
        return self.ncores * self.TPC

    @property
    def TT(self):
        return self.TPC // 128  # token tiles per core

    @property
    def M(self):
        return self.N // 128    # global token groups (= ncores*TT)

    @property
    def DCH(self):
        return self.D // 128

    @property
    def HCH(self):
        return self.H // 128

    @property
    def NCHUNK(self):
        return self.CAPP // self.CCHUNK

    @property
    def DHN(self):
        return min(512, self.D)  # matmul2 free-dim chunk

    @property
    def NDH(self):
        return self.D // self.DHN


def build(cfg: Cfg, dbg: bool = False):
    """Build the Bacc program. Returns nc."""
    E, K, D, H = cfg.E, cfg.K, cfg.D, cfg.H
    TPC, TT, M, N = cfg.TPC, cfg.TT, cfg.M, cfg.N
    DCH, HCH = cfg.DCH, cfg.HCH
    CAP, CAPP, CCHUNK, NCHUNK = cfg.cap, cfg.CAPP, cfg.CCHUNK, cfg.NCHUNK
    DHN, NDH = cfg.DHN, cfg.NDH
    NC = cfg.ncores
    assert E == NC == 8 and K == 2
    assert TPC % 128 == 0 and D % 128 == 0 and H % 128 == 0
    assert CCHUNK % 128 == 0 and CAPP % CCHUNK == 0 and CAP < CAPP
    NSLOT = E * CAPP  # 20480
    # expert-0 slot CAPP-1 is never dispatched (cap < CAPP) -> zero row
    ZSLOT = CAPP - 1

    nc = bacc.Bacc("TRN2", target_bir_lowering=False, debug=False,
                   num_devices=NC)

    # ---- external inputs (per-core staged by host) ----
    xT_shard = nc.dram_tensor("xT_shard", [D, TPC], FP32, kind="ExternalInput")
    x_bf16 = nc.dram_tensor("x_bf16", [N, D], BF16, kind="ExternalInput")
    rank_in = nc.dram_tensor("rank_in", [128, 1], FP32, kind="ExternalInput")
    Wr_in = nc.dram_tensor("Wr_in", [128, DCH, E], FP32, kind="ExternalInput")
    br_in = nc.dram_tensor("br_in", [1, E], FP32, kind="ExternalInput")
    W1_in = nc.dram_tensor("W1_in", [128, DCH, H], BF16, kind="ExternalInput")
    W2_in = nc.dram_tensor("W2_in", [128, HCH, D], BF16, kind="ExternalInput")
    b1_in = nc.dram_tensor("b1_in", [128, HCH], FP32, kind="ExternalInput")
    b2_in = nc.dram_tensor("b2_in", [1, D], BF16, kind="ExternalInput")
    ltri_in = nc.dram_tensor("ltri_in", [128, 128], BF16, kind="ExternalInput")

    # ---- external output ----
    y_out = nc.dram_tensor("y_out", [TPC, D], FP32, kind="ExternalOutput")

    with tile.TileContext(nc) as tc:
        rank_sp = nc.partition_id()

        keepp_cm = tc.tile_pool(name="keepp", bufs=1)
        keepp = keepp_cm.__enter__()
        dramp_cm = tc.tile_pool(name="dramp", bufs=1, space="DRAM")
        dramp = dramp_cm.__enter__()
        routing_local = dramp.tile([128, TT * 4], FP32, tag="routing_local")
        routing_all = dramp.tile([NC, 128, TT * 4], FP32, tag="routing_all",
                                 addr_space="Shared")
        posx_local = dramp.tile([128, M], FP32, tag="posx_local")
        posx_all = dramp.tile([NC, 128, M], FP32, tag="posx_all",
                              addr_space="Shared")
        disp_rec = dramp.tile([CAPP + 128, 64], FP32, tag="disp_rec")
        scat_bounce = dramp.tile([K, 128, M], I16, tag="scat_bounce")
        out_e = dramp.tile([CAPP, D], BF16, tag="out_e")
        all_out = dramp.tile([NC, CAPP, D], BF16, tag="all_out",
                             addr_space="Shared")
        flat_bounce = dramp.tile([128, K, TT], I16, tag="flat_bounce")

        cpool_cm = tc.tile_pool(name="const", bufs=1)
        cpool = cpool_cm.__enter__()

        wts1_cm = tc.tile_pool(name="wts1", bufs=1)
        wts1 = wts1_cm.__enter__()

        # ---------- phase 0: router-critical DMAs first ----------
        xp_cm = tc.tile_pool(name="xp", bufs=1)
        xp = xp_cm.__enter__()
        # x shard, transposed, fp32: [128, DCH, TPC] (64KB/partition),
        # loaded in quarters so the router matmuls pipeline with the load
        xsb = xp.tile([128, DCH, TPC], FP32, tag="xsb")
        xsrc = xT_shard.rearrange("(c p) t -> p c t", p=128)
        TQ = TPC // 4
        for q4 in range(4):
            nc.sync.dma_start(xsb[:, :, q4 * TQ:(q4 + 1) * TQ],
                              xsrc[:, :, q4 * TQ:(q4 + 1) * TQ])

        W1s = wts1.tile([128, DCH, H], BF16, tag="W1s")
        b1s = wts1.tile([128, HCH], FP32, tag="b1s")
        b2s = wts1.tile([1, D], BF16, tag="b2s")

        # ---------- constants ----------
        ones128f = cpool.tile([128, 128], FP32, tag="ones128f")
        nc.vector.memset(ones128f, 1.0)
        ones1f = cpool.tile([1, 128], FP32, tag="ones1f")
        nc.vector.memset(ones1f, 1.0)
        ones1b = cpool.tile([1, 128], BF16, tag="ones1b")
        nc.vector.memset(ones1b, 1.0)
        zerosM = cpool.tile([128, M], FP32, tag="zerosM")
        nc.vector.memset(zerosM, 0.0)
        bigT = cpool.tile([128, M], FP32, tag="bigT")
        nc.vector.memset(bigT, float(CAPP))     # scatter dump row
        bigP = cpool.tile([128, M], FP32, tag="bigP")
        nc.vector.memset(bigP, float(NSLOT))    # dropped marker in posx
        zslotc = cpool.tile([128, TT], FP32, tag="zslotc")
        nc.vector.memset(zslotc, float(ZSLOT))
        one_i = cpool.tile([128, 1], I32, tag="one_i")
        nc.vector.memset(one_i, 1)
        ebaseE = cpool.tile([128, E], FP32, tag="ebaseE")
        for e in range(E):
            nc.vector.memset(ebaseE[:, e:e + 1], float(e * CAPP))
        rankf = cpool.tile([128, 1], FP32, tag="rankf")
        nc.sync.dma_start(rankf, rank_in[:, :])
        ltri = cpool.tile([128, 128], BF16, tag="ltri")
        nc.sync.dma_start(ltri, ltri_in[:, :])
        wr_sb = cpool.tile([128, DCH, E], FP32, tag="wr")
        nc.sync.dma_start(wr_sb, Wr_in[:, :, :])
        br_sb = cpool.tile([1, E], FP32, tag="br")
        nc.sync.dma_start(br_sb, br_in[:, :])
        # token-id iota: tokid[p, m] = m*128 + p
        tokid_i = cpool.tile([128, M], I32, tag="tokid_i")
        nc.gpsimd.iota(tokid_i, pattern=[[128, M]], base=0,
                       channel_multiplier=1)
        tokid_f = cpool.tile([128, M], FP32, tag="tokid_f")
        nc.vector.tensor_copy(tokid_f, tokid_i)

        # ---------- P1: router (own shard, fp32) ----------
        with tc.tile_pool(name="psr", bufs=2, space="PSUM") as psr:
            E_sb = xp.tile([128, TT, E], FP32, tag="E_sb")
            M8 = xp.tile([128, TT, 8], FP32, tag="M8")
            I8 = xp.tile([128, TT, 8], U16, tag="I8")
            Z_sb = xp.tile([128, TT], FP32, tag="Z_sb")
            RT_loc = xp.tile([128, TT, 4], FP32, tag="RT_loc")

            for t in range(TT):
                ps = psr.tile([128, E], FP32, tag="psr")
                for dch in range(DCH):
                    nc.tensor.matmul(
                        ps, lhsT=xsb[:, dch, t * 128:(t + 1) * 128],
                        rhs=wr_sb[:, dch, :],
                        start=(dch == 0), stop=False)
                nc.tensor.matmul(ps, lhsT=ones1f, rhs=br_sb[:, :],
                                 start=False, stop=True)
                nc.scalar.activation(E_sb[:, t, :], ps, AF.Exp)
                nc.vector.max(M8[:, t, :], E_sb[:, t, :])
                nc.vector.max_index(I8[:, t, :], M8[:, t, :], E_sb[:, t, :])
            nc.vector.tensor_reduce(Z_sb, E_sb, AX.X, ALU.add)
            rZ = xp.tile([128, TT], FP32, tag="rZ")
            nc.vector.reciprocal(rZ, Z_sb)
            nc.vector.tensor_copy(RT_loc[:, :, 0], I8[:, :, 0])
            nc.vector.tensor_tensor(RT_loc[:, :, 1], M8[:, :, 0], rZ,
                                    ALU.mult)
            nc.vector.tensor_copy(RT_loc[:, :, 2], I8[:, :, 1])
            nc.vector.tensor_tensor(RT_loc[:, :, 3], M8[:, :, 1], rZ,
                                    ALU.mult)
            nc.sync.dma_start(routing_local,
                              RT_loc.rearrange("p t q -> p (t q)"))

        # non-router-critical loads, issued after the router's in program
        # order so xsb wins the DMA queue
        nc.sync.dma_start(W1s, W1_in[:, :, :])
        nc.sync.dma_start(b1s, b1_in[:, :])
        nc.sync.dma_start(b2s, b2_in[:, :])
        # zero disp_rec (padded slots must read token 0 / gate 0)
        zbig = xp.tile([128, 1344], FP32, tag="zbig")
        nc.vector.memset(zbig, 0.0)
        drv = disp_rec.rearrange("(p w) c -> p (w c)", p=128)
        nc.sync.dma_start(drv, zbig)

        nc.gpsimd.collective_compute(
            "AllGather", ALU.bypass,
            replica_groups=[list(range(NC))],
            ins=[routing_local.opt()], outs=[routing_all.opt()])
        xp_cm.__exit__(None, None, None)  # free xsb

        # W2 load starts once xsb's SBUF is free
        wts2_cm = tc.tile_pool(name="wts2", bufs=1)
        wts2 = wts2_cm.__enter__()
        W2s = wts2.tile([128, HCH, D], BF16, tag="W2s")
        nc.sync.dma_start(W2s, W2_in[:, :, :])
        # b2 broadcast to all 128 partitions via rank-1 matmul
        b2bc = wts1.tile([128, D], FP32, tag="b2bc")

        selctx = [tc.tile_pool(name="sel", bufs=1),
                  tc.tile_pool(name="selsm", bufs=1),
                  tc.tile_pool(name="pscnt", bufs=2, space="PSUM")]
        sel, ssm, pscnt = [c.__enter__() for c in selctx]
        if True:
            for dh in range(NDH):
                pb = pscnt.tile([128, DHN], FP32, tag="pscnt")
                nc.tensor.matmul(pb, lhsT=ones1b,
                                 rhs=b2s[:, dh * DHN:(dh + 1) * DHN],
                                 start=True, stop=True)
                nc.vector.tensor_copy(b2bc[:, dh * DHN:(dh + 1) * DHN], pb)

            # ---------- P2: per-expert (sharded) selection ----------
            # RTA[p, m, q], token n = m*128+p, q = (i1, g1, i2, g2)
            RTA = sel.tile([128, M, 4], FP32, tag="RTA")
            nc.sync.dma_start(
                RTA, routing_all.rearrange("r p (t q) -> p r t q", q=4))
            i1f = RTA[:, :, 0]
            g1f = RTA[:, :, 1]
            i2f = RTA[:, :, 2]
            g2f = RTA[:, :, 3]

            # A_own[token] = gate if token routed to OUR expert, else 0
            A_own = sel.tile([128, M], FP32, tag="A_own")
            tmpM = sel.tile([128, M], FP32, tag="tmpM")
            nc.vector.scalar_tensor_tensor(
                A_own, i1f, rankf, g1f, op0=ALU.is_equal, op1=ALU.mult)
            nc.vector.scalar_tensor_tensor(
                tmpM, i2f, rankf, g2f, op0=ALU.is_equal, op1=ALU.mult)
            nc.vector.tensor_tensor(A_own, A_own, tmpM, ALU.add)

            big = sel.tile([128, M], FP32, tag="big")
            cnt1 = sel.tile([128, 1], FP32, tag="cnt1")
            Ktgt = sel.tile([128, 1], FP32, tag="Ktgt")
            lo = sel.tile([128, 1], I32, tag="lo")
            hi = sel.tile([128, 1], I32, tag="hi")
            mid = sel.tile([128, 1], I32, tag="mid")
            condi = sel.tile([128, 1], I32, tag="condi")

            nc.vector.tensor_scalar(big, A_own, 0.0, None, op0=ALU.is_gt)
            nc.vector.tensor_reduce(cnt1, big, AX.X, ALU.add)
            pc0 = pscnt.tile([128, 1], FP32, tag="pscnt")
            nc.tensor.matmul(pc0, lhsT=ones128f, rhs=cnt1, start=True,
                             stop=True)
            nc.vector.tensor_scalar(Ktgt, pc0, float(CAP), None, op0=ALU.min)

            nc.vector.memset(lo, 0)
            nc.vector.memset(hi, 0x3F800000)
            for it in range(cfg.NBIS):
                nc.vector.tensor_tensor(mid, lo, hi, ALU.add)
                nc.vector.tensor_tensor(mid, mid, one_i,
                                        ALU.logical_shift_right)
                midf = mid.bitcast(FP32)
                nc.vector.tensor_scalar(big, A_own, midf, None,
                                        op0=ALU.is_gt)
                nc.vector.tensor_reduce(cnt1, big, AX.X, ALU.add)
                pc = pscnt.tile([128, 1], FP32, tag="pscnt")
                nc.tensor.matmul(pc, lhsT=ones128f, rhs=cnt1, start=True,
                                 stop=True)
                nc.vector.tensor_tensor(condi, pc, Ktgt, ALU.is_ge)
                nc.vector.copy_predicated(lo, condi, mid)
                nc.vector.tensor_tensor(condi, pc, Ktgt, ALU.is_lt)
                nc.vector.copy_predicated(hi, condi, mid)

            thrf = lo.bitcast(FP32)
            keepf = sel.tile([128, M], FP32, tag="keepf")
            nc.vector.tensor_scalar(keepf, A_own, thrf, None, op0=ALU.is_gt)
            keepi = sel.tile([128, M], I32, tag="keepi")
            nc.vector.tensor_copy(keepi, keepf)

            # positions within our expert buffer: slot order = p*M + m
            rp = sel.tile([128, M], FP32, tag="rp")
            nc.vector.tensor_tensor_scan(
                rp, keepf, zerosM, initial=0.0, op0=ALU.add, op1=ALU.add)
            totb = sel.tile([128, 1], BF16, tag="totb")
            nc.vector.tensor_copy(totb, rp[:, M - 1:M])
            pe_x = pscnt.tile([128, 1], FP32, tag="pscnt")
            nc.tensor.matmul(pe_x, lhsT=ltri, rhs=totb, start=True, stop=True)
            excl = sel.tile([128, 1], FP32, tag="excl")
            nc.vector.tensor_copy(excl, pe_x)
            pos = sel.tile([128, M], FP32, tag="pos")
            nc.vector.scalar_tensor_tensor(
                pos, rp, excl, keepf, op0=ALU.add, op1=ALU.subtract)

            # share (pos or BIG) with every core for the combine phase
            posx = sel.tile([128, M], FP32, tag="posx")
            nc.vector.tensor_copy(posx, bigP)
            nc.vector.copy_predicated(posx, keepi, pos)
            nc.sync.dma_start(posx_local, posx)
            nc.gpsimd.collective_compute(
                "AllGather", ALU.bypass,
                replica_groups=[list(range(NC))],
                ins=[posx_local.opt()], outs=[posx_all.opt()])

            # dispatch scatter indices (own expert only; others -> dump row)
            sci16 = sel.tile([128, K, M], I16, tag="sci16")
            elig = sel.tile([128, M], FP32, tag="elig")
            eligi = sel.tile([128, M], I32, tag="eligi")
            offd = sel.tile([128, M], FP32, tag="offd")
            for k in range(K):
                ikf = i1f if k == 0 else i2f
                nc.vector.scalar_tensor_tensor(
                    elig, ikf, rankf, keepf, op0=ALU.is_equal, op1=ALU.mult)
                nc.vector.tensor_copy(eligi, elig)
                nc.vector.tensor_copy(offd, bigT)
                nc.vector.copy_predicated(offd, eligi, pos)
                nc.vector.tensor_copy(sci16[:, k, :], offd)
            nc.sync.dma_start(scat_bounce.rearrange("k p m -> p k m"), sci16)

            # combine flat idx for own tokens, from the AllGathered
            # per-expert positions: raw = pos[e_k] + e_k*CAPP, dropped->BIG
            PXA = sel.tile([128, E, TT], FP32, tag="PXA")
            nc.sync.dma_start(
                PXA,
                posx_all.rearrange("r p m -> p r m")[
                    :, :, bass.ds(rank_sp * TT, TT)])
            nc.vector.tensor_tensor(
                PXA, PXA, ebaseE.unsqueeze(2).broadcast_to((128, E, TT)),
                ALU.add)
            ci16 = sel.tile([128, K, TT], I16, tag="ci16")
            rawk = sel.tile([128, TT], FP32, tag="rawk")
            tmpT = sel.tile([128, TT], FP32, tag="tmpT")
            keepT = sel.tile([128, TT], I32, tag="keepT")
            fck = sel.tile([128, TT], FP32, tag="fck")
            own0 = bass.ds(rank_sp * TT, TT)
            for k in range(K):
                ikf_o = (i1f if k == 0 else i2f)[:, own0]
                first = True
                for e in range(E):
                    dst = rawk if first else tmpT
                    nc.vector.scalar_tensor_tensor(
                        dst, ikf_o, float(e), PXA[:, e, :],
                        op0=ALU.is_equal, op1=ALU.mult)
                    if not first:
                        nc.vector.tensor_tensor(rawk, rawk, tmpT, ALU.add)
                    first = False
                # kept iff raw's pos-part < NSLOT (dropped marker >= NSLOT)
                nc.vector.tensor_scalar(tmpT, rawk, float(NSLOT), None,
                                        op0=ALU.is_lt)
                nc.vector.tensor_copy(keepT, tmpT)
                nc.vector.tensor_copy(fck, zslotc)
                nc.vector.copy_predicated(fck, keepT, rawk)
                nc.vector.tensor_copy(ci16[:, k, :], fck)
            nc.sync.dma_start(flat_bounce, ci16)
            # scatter (tok, gate) records into disp_rec rows (8B payload,
            # 256B row stride)
            pay = sel.tile([128, M, 2], FP32, tag="pay")
            sidx = sel.tile([128, M * 8], I16, tag="sidx")
            for k in range(K):
                gkf = g1f if k == 0 else g2f
                nc.vector.tensor_copy(pay[:, :, 0], tokid_f)
                nc.vector.tensor_copy(pay[:, :, 1], gkf)
                # idx item i = m*128+p -> [p%16, m*8+p//16]; fold once into
                # partitions 0:16, then replicate via cheap SBUF copies
                nc.sync.dma_start(
                    sidx[0:16, :].rearrange("p (m h) -> p m h", h=8),
                    scat_bounce[k, :, :].rearrange(
                        "(ph pl) m -> pl m ph", pl=16))
                for g in range(1, 8):
                    nc.sync.dma_start(sidx[g * 16:(g + 1) * 16, :],
                                      sidx[0:16, :])
                SC = min(8, M)  # m-groups per scatter (<=1024 items)
                for m0 in range(0, M, SC):
                    nc.gpsimd.dma_scatter_add(
                        out_ap=disp_rec[:, 0:2],
                        in_ap=pay[:, m0:m0 + SC, :],
                        idxs_ap=sidx[:, m0 * 8:(m0 + SC) * 8],
                        num_idxs=SC * 128,
                        num_idxs_reg=SC * 128,
                        elem_size=2,
                        elem_step=64)

            # combine gather idx tile [16, K*TT*8] -> [128, K*TT*8] padded
            cidx = keepp.tile([128, K * TT * 8], I16, tag="cidx")
            nc.sync.dma_start(
                cidx[0:16, :],
                flat_bounce.rearrange("(ph pl) k t -> pl k t ph", pl=16))
            for g in range(1, 8):
                nc.sync.dma_start(cidx[g * 16:(g + 1) * 16, :], cidx[0:16, :])

            # dispatch idx tile + slot gates (local per-expert table)
            gdisp = keepp.tile([128, CAPP // 128], FP32, tag="gdisp")
            nc.sync.dma_start(
                gdisp,
                disp_rec[0:CAPP, :].rearrange(
                    "(c s) w -> s c w", s=128)[:, :, 1])
            tokf16 = sel.tile([16, CAPP // 16], FP32, tag="tokf16")
            nc.sync.dma_start(
                tokf16,
                disp_rec[0:CAPP, :].rearrange(
                    "(c q) w -> q c w", q=16)[:, :, 0])
            dIdx = keepp.tile([128, CAPP // 16], I16, tag="dIdx")
            nc.vector.tensor_copy(dIdx[0:16, :], tokf16)
            for g in range(1, 8):
                nc.sync.dma_start(dIdx[g * 16:(g + 1) * 16, :], dIdx[0:16, :])

        # ---------- P3: expert FFN ----------
        for c_ in reversed(selctx):
            c_.__exit__(None, None, None)
        with tc.tile_pool(name="ffn", bufs=2) as ffn, \
             tc.tile_pool(name="ht", bufs=1) as htp, \
             tc.tile_pool(name="ps1", bufs=2, space="PSUM") as ps1p, \
             tc.tile_pool(name="ps2", bufs=2, space="PSUM") as ps2p:

            for c in range(NCHUNK):
                xTg = ffn.tile([128, DCH, CCHUNK], BF16, tag="xTg")
                nc.gpsimd.dma_gather(
                    out_ap=xTg,
                    in_ap=x_bf16[:, :],
                    idxs_ap=dIdx[:, c * (CCHUNK // 16):
                                 (c + 1) * (CCHUNK // 16)],
                    num_idxs=CCHUNK,
                    num_idxs_reg=CCHUNK,
                    elem_size=D,
                    transpose=True)
                hT = htp.tile([128, HCH, CCHUNK], BF16, tag="hT")
                for j in range(HCH):
                    ps1 = ps1p.tile([128, CCHUNK], FP32, tag="ps1")
                    for dch in range(DCH):
                        nc.tensor.matmul(
                            ps1, lhsT=W1s[:, dch, j * 128:(j + 1) * 128],
                            rhs=xTg[:, dch, :],
                            start=(dch == 0), stop=(dch == DCH - 1))
                    sgt = ffn.tile([128, CCHUNK], FP32, tag="sgt")
                    nc.scalar.activation(sgt, ps1, AF.Sigmoid,
                                         bias=b1s[:, j:j + 1])
                    nc.vector.scalar_tensor_tensor(
                        hT[:, j, :], ps1, b1s[:, j:j + 1], sgt,
                        op0=ALU.add, op1=ALU.mult)
                for cs in range(CCHUNK // 128):
                    col = c * (CCHUNK // 128) + cs
                    # b2g = b2 * gate (per-token row scalar)
                    b2g = ffn.tile([128, D], FP32, tag="b2g")
                    nc.vector.tensor_scalar(
                        b2g, b2bc, gdisp[:, col:col + 1], None, op0=ALU.mult)
                    osb = ffn.tile([128, D], BF16, tag="osb")
                    for dh in range(NDH):
                        ps2 = ps2p.tile([128, DHN], FP32, tag="ps2")
                        for j in range(HCH):
                            nc.tensor.matmul(
                                ps2,
                                lhsT=hT[:, j, cs * 128:(cs + 1) * 128],
                                rhs=W2s[:, j, dh * DHN:(dh + 1) * DHN],
                                start=(j == 0), stop=(j == HCH - 1))
                        # osb = ps2*gate + b2*gate
                        nc.vector.scalar_tensor_tensor(
                            osb[:, dh * DHN:(dh + 1) * DHN], ps2,
                            gdisp[:, col:col + 1],
                            b2g[:, dh * DHN:(dh + 1) * DHN],
                            op0=ALU.mult, op1=ALU.add)
                    nc.sync.dma_start(
                        out_e[col * 128:(col + 1) * 128, :], osb)

        nc.gpsimd.collective_compute(
            "AllGather", ALU.bypass,
            replica_groups=[list(range(NC))],
            ins=[out_e.opt()], outs=[all_out.opt()])

        wts2_cm.__exit__(None, None, None)
        wts1_cm.__exit__(None, None, None)

        # ---------- P4: combine own shard ----------
        with tc.tile_pool(name="comb", bufs=2) as comb:
            GC = min(8, TT)  # t-tiles per gather (<=1024 idxs)
            gk_tiles = []
            for k in range(K):
                gk = comb.tile([128, TT, D], BF16, tag=f"gk{k}")
                gk_tiles.append(gk)
            for t0 in range(0, TT, GC):
                for k in range(K):
                    nc.gpsimd.dma_gather(
                        out_ap=gk_tiles[k][:, t0:t0 + GC, :],
                        idxs_ap=cidx[:, k * TT * 8 + t0 * 8:
                                     k * TT * 8 + (t0 + GC) * 8],
                        in_ap=all_out.rearrange("r c d -> (r c) d"),
                        num_idxs=GC * 128,
                        num_idxs_reg=GC * 128,
                        elem_size=D,
                        transpose=False)
                for t in range(t0, t0 + GC):
                    ysb = comb.tile([128, D], FP32, tag="ysb")
                    nc.vector.tensor_tensor(ysb, gk_tiles[0][:, t, :],
                                            gk_tiles[1][:, t, :], ALU.add)
                    nc.sync.dma_start(y_out[t * 128:(t + 1) * 128, :], ysb)

        cpool_cm.__exit__(None, None, None)
        keepp_cm.__exit__(None, None, None)
        dramp_cm.__exit__(None, None, None)

    nc.compile()
    return nc


# ---------------- host-side staging ----------------

def stage_inputs(cfg: Cfg, x, Wr, br, W1, b1, W2, b2):
    """x: [N, D] fp32; returns list of per-core input dicts."""
    E, D, H, TPC, NC = cfg.E, cfg.D, cfg.H, cfg.TPC, cfg.ncores
    DCH, HCH = cfg.DCH, cfg.HCH
    x = np.ascontiguousarray(x, np.float32)
    x_bf = x.astype(bfloat16_np())
    ltri = np.tril(np.ones((128, 128), np.float32), -1).astype(bfloat16_np())
    in_maps = []
    for r in range(NC):
        shard = x[r * TPC:(r + 1) * TPC]
        m = {
            "xT_shard": np.ascontiguousarray(shard.T),
            "x_bf16": x_bf,
            "rank_in": np.full((128, 1), r, np.float32),
            "Wr_in": np.ascontiguousarray(
                Wr.reshape(DCH, 128, E).transpose(1, 0, 2)).astype(np.float32),
            "br_in": br.reshape(1, E).astype(np.float32),
            "W1_in": np.ascontiguousarray(
                W1[r].reshape(DCH, 128, H).transpose(1, 0, 2)
            ).astype(bfloat16_np()),
            "W2_in": np.ascontiguousarray(
                W2[r].reshape(HCH, 128, D).transpose(1, 0, 2)
            ).astype(bfloat16_np()),
            "b1_in": np.ascontiguousarray(
                b1[r].reshape(HCH, 128).T).astype(np.float32),
            "b2_in": b2[r].reshape(1, D).astype(np.float32).astype(
                bfloat16_np()),
            "ltri_in": ltri,
        }
        in_maps.append(m)
    return in_maps


def bfloat16_np():
    import ml_dtypes
    return ml_dtypes.bfloat16


def unshard(cfg: Cfg, results, B, T):
    ys = [results[r]["y_out"] for r in range(cfg.ncores)]
    y = np.concatenate(ys, axis=0)
    return y.reshape(B, T, cfg.D)


# ---------------- problem binding ----------------

import math as _math

B, T = 8, 2048
_N = B * T
_D = 1024
_CAP = int(_math.ceil(1.2 * _N / 8))  # 2458

_CACHE = {}


def _get_nc():
    if "nc" not in _CACHE:
        cfg = Cfg(D=_D, H=4096, TPC=_N // 8, cap=_CAP, CAPP=2560, CCHUNK=512)
        _CACHE["cfg"] = cfg
        _CACHE["nc"] = build(cfg)
    return _CACHE["cfg"], _CACHE["nc"]


def kernel(x_btd, Wr, br, W1, b1, W2, b2):
    from concourse.bass_utils import run_bass_kernel_spmd

    cfg, nc = _get_nc()
    x = np.ascontiguousarray(np.asarray(x_btd), np.float32).reshape(_N, _D)
    in_maps = stage_inputs(
        cfg, x, np.asarray(Wr), np.asarray(br), np.asarray(W1),
        np.asarray(b1), np.asarray(W2), np.asarray(b2))
    res = run_bass_kernel_spmd(nc, in_maps, list(range(8)))
    ys = [res.results[r]["y_out"] for r in range(8)]
    y = np.concatenate(ys, axis=0).astype(np.float32)
    return y.reshape(B, T, _D)


# revision 47
# speedup vs baseline: 4.5916x; 4.5916x over previous
"""TRN2 Bass kernel for nn_MoEPositionwiseFFN: kernel(**inputs) -> np.ndarray.

Self-contained: builds (and caches) an 8-core SPMD Bass/Tile NEFF
implementing the MoE positionwise FFN (router + top-2 + capacity drop +
expert FFN + gated combine), shards the full inputs across the 8
NeuronCores (expert-parallel weights, data-parallel tokens, replicated
bf16 x for local dispatch gathers), runs it, and reassembles the full
[B, T, D] output.

Layout note: dispatch/combine slots use a chunk-major flat coordinate
  off2 = (slot//CCHUNK)*(NC*CCHUNK) + e*CCHUNK + slot%CCHUNK
so that the per-FFN-chunk AllGather output slices are contiguous in
all_out, letting the collective overlap the remaining FFN chunks.
"""

import sys

for _p in ("/opt/trn_rl_repo", "/opt/pypackages"):
    if _p not in sys.path:
        sys.path.insert(0, _p)


from dataclasses import dataclass

import numpy as np

import concourse.bass as bass
import concourse.bacc as bacc
import concourse.tile as tile
import concourse.mybir as mybir

FP32 = mybir.dt.float32
BF16 = mybir.dt.bfloat16
I32 = mybir.dt.int32
I16 = mybir.dt.int16
U16 = mybir.dt.uint16
AF = mybir.ActivationFunctionType
ALU = mybir.AluOpType
AX = mybir.AxisListType


@dataclass
class Cfg:
    ncores: int = 8
    E: int = 8
    K: int = 2
    D: int = 1024
    H: int = 4096
    TPC: int = 2048          # tokens per core
    cap: int = 2458          # reference capacity
    CAPP: int = 2560         # padded capacity (multiple of CCHUNK, > cap)
    CCHUNK: int = 512        # FFN token chunk (multiple of 128, <= 512)
    NBIS: int = 30           # bisection iterations (covers bits 0..0x3F800000)

    @property
    def N(self):
        return self.ncores * self.TPC

    @property
    def TT(self):
        return self.TPC // 128  # token tiles per core

    @property
    def M(self):
        return self.N // 128    # global token groups (= ncores*TT)

    @property
    def DCH(self):
        return self.D // 128

    @property
    def HCH(self):
        return self.H // 128

    @property
    def NCHUNK(self):
        return self.CAPP // self.CCHUNK

    @property
    def DHN(self):
        return min(512, self.D)  # matmul2 free-dim chunk

    @property
    def NDH(self):
        return self.D // self.DHN


def build(cfg: Cfg, dbg: bool = False):
    """Build the Bacc program. Returns nc."""
    E, K, D, H = cfg.E, cfg.K, cfg.D, cfg.H
    TPC, TT, M, N = cfg.TPC, cfg.TT, cfg.M, cfg.N
    DCH, HCH = cfg.DCH, cfg.HCH
    CAP, CAPP, CCHUNK, NCHUNK = cfg.cap, cfg.CAPP, cfg.CCHUNK, cfg.NCHUNK
    DHN, NDH = cfg.DHN, cfg.NDH
    NC = cfg.ncores
    assert E == NC == 8 and K == 2
    assert TPC % 128 == 0 and D % 128 == 0 and H % 128 == 0
    assert CCHUNK % 128 == 0 and CAPP % CCHUNK == 0 and CAP < CAPP
    NSLOT = E * CAPP  # 20480
    # expert-0 slot CAPP-1 is never dispatched (cap < CAPP) -> zero row
    ZSLOT = CAPP - 1

    nc = bacc.Bacc("TRN2", target_bir_lowering=False, debug=False,
                   num_devices=NC)

    # ---- external inputs (per-core staged by host) ----
    xT_shard = nc.dram_tensor("xT_shard", [D, TPC], FP32, kind="ExternalInput")
    x_bf16 = nc.dram_tensor("x_bf16", [N, D], BF16, kind="ExternalInput")
    rank_in = nc.dram_tensor("rank_in", [128, 1], FP32, kind="ExternalInput")
    Wr_in = nc.dram_tensor("Wr_in", [128, DCH, E], FP32, kind="ExternalInput")
    br_in = nc.dram_tensor("br_in", [1, E], FP32, kind="ExternalInput")
    W1_in = nc.dram_tensor("W1_in", [128, DCH, H], BF16, kind="ExternalInput")
    W2_in = nc.dram_tensor("W2_in", [128, HCH, D], BF16, kind="ExternalInput")
    b1_in = nc.dram_tensor("b1_in", [128, HCH], FP32, kind="ExternalInput")
    b2_in = nc.dram_tensor("b2_in", [1, D], BF16, kind="ExternalInput")
    ltri_in = nc.dram_tensor("ltri_in", [128, 128], BF16, kind="ExternalInput")

    # ---- external output ----
    y_out = nc.dram_tensor("y_out", [TPC, D], FP32, kind="ExternalOutput")
    if dbg:
        dbg_rt = nc.dram_tensor("dbg_rt", [128, TT * 4], FP32,
                                kind="ExternalOutput")
        dbg_thr = nc.dram_tensor("dbg_thr", [128, 1], I32,
                                 kind="ExternalOutput")
        dbg_pos = nc.dram_tensor("dbg_pos", [128, M], FP32,
                                 kind="ExternalOutput")
        dbg_keep = nc.dram_tensor("dbg_keep", [128, M], FP32,
                                  kind="ExternalOutput")
        dbg_sci = nc.dram_tensor("dbg_sci", [128, M], I16,
                                 kind="ExternalOutput")
        dbg_gd = nc.dram_tensor("dbg_gd", [128, CAPP // 128], FP32,
                                kind="ExternalOutput")
        dbg_tok = nc.dram_tensor("dbg_tok", [16, CAPP // 16], FP32,
                                 kind="ExternalOutput")
        dbg_ci = nc.dram_tensor("dbg_ci", [128, K * TT], I16,
                                kind="ExternalOutput")
        dbg_pxa = nc.dram_tensor("dbg_pxa", [128, E * TT], FP32,
                                 kind="ExternalOutput")
        dbg_xtg = nc.dram_tensor("dbg_xtg", [128, DCH * CCHUNK], BF16,
                                 kind="ExternalOutput")

    with tile.TileContext(nc) as tc:
        rank_sp = nc.partition_id()

        keepp_cm = tc.tile_pool(name="keepp", bufs=1)
        keepp = keepp_cm.__enter__()
        dramp_cm = tc.tile_pool(name="dramp", bufs=1, space="DRAM")
        dramp = dramp_cm.__enter__()
        routing_local = dramp.tile([128, TT * 4], FP32, tag="routing_local")
        routing_all = dramp.tile([NC, 128, TT * 4], FP32, tag="routing_all",
                                 addr_space="Shared")
        posx_local = dramp.tile([128, M], FP32, tag="posx_local")
        posx_all = dramp.tile([NC, 128, M], FP32, tag="posx_all",
                              addr_space="Shared")
        disp_rec = dramp.tile([CAPP + 128, 64], FP32, tag="disp_rec")
        disp_recB = dramp.tile([CAPP + 128, 64], FP32, tag="disp_recB")
        scat_bounce = dramp.tile([128, M], I16, tag="scat_bounce")
        out_e = dramp.tile([CAPP, D], BF16, tag="out_e")
        all_out = dramp.tile([NC, CAPP, D], BF16, tag="all_out",
                             addr_space="Shared")
        flat_bounce = dramp.tile([128, K, TT], I16, tag="flat_bounce")

        cpool_cm = tc.tile_pool(name="const", bufs=1)
        cpool = cpool_cm.__enter__()

        wts1_cm = tc.tile_pool(name="wts1", bufs=1)
        wts1 = wts1_cm.__enter__()

        # ---------- phase 0: router-critical DMAs first ----------
        xp_cm = tc.tile_pool(name="xp", bufs=1)
        xp = xp_cm.__enter__()
        # x shard, transposed, fp32: [128, DCH, TPC] (64KB/partition),
        # loaded in quarters (separate tiles: precise deps) so the router
        # matmuls pipeline with the load
        xsrc = xT_shard.rearrange("(c p) t -> p c t", p=128)
        TQ = TPC // 4
        xsbq = []
        for q4 in range(4):
            xq = xp.tile([128, DCH, TQ], FP32, tag=f"xsb{q4}")
            xsbq.append(xq)

        def load_xq(q4):
            nc.sync.dma_start(xsbq[q4], xsrc[:, :, q4 * TQ:(q4 + 1) * TQ])

        load_xq(0)
        load_xq(1)

        W1s = wts1.tile([128, DCH, H], BF16, tag="W1s")
        b1s = wts1.tile([128, HCH], FP32, tag="b1s")
        b2s = wts1.tile([1, D], BF16, tag="b2s")
        nc.sync.dma_start(b2s, b2_in[:, :])  # tiny; read by b2bc matmul below

        # ---------- constants ----------
        ones128f = cpool.tile([128, 128], FP32, tag="ones128f")
        nc.vector.memset(ones128f, 1.0)
        ones1f = cpool.tile([1, 128], FP32, tag="ones1f")
        nc.vector.memset(ones1f, 1.0)
        ones1b = cpool.tile([1, 128], BF16, tag="ones1b")
        nc.vector.memset(ones1b, 1.0)
        zerosM = cpool.tile([128, M], FP32, tag="zerosM")
        nc.vector.memset(zerosM, 0.0)
        bigT = cpool.tile([128, M], FP32, tag="bigT")
        nc.vector.memset(bigT, float(CAPP))     # scatter dump row
        bigP = cpool.tile([128, M], FP32, tag="bigP")
        nc.vector.memset(bigP, float(NSLOT))    # dropped marker in posx
        zslotc = cpool.tile([128, TT], FP32, tag="zslotc")
        nc.vector.memset(zslotc, float(ZSLOT))
        one_i = cpool.tile([128, 1], I32, tag="one_i")
        nc.vector.memset(one_i, 1)
        ebaseE = cpool.tile([128, E], FP32, tag="ebaseE")
        for e in range(E):
            nc.vector.memset(ebaseE[:, e:e + 1], float(e * CAPP))
        rankf = cpool.tile([128, 1], FP32, tag="rankf")
        nc.sync.dma_start(rankf, rank_in[:, :])
        ltri = cpool.tile([128, 128], BF16, tag="ltri")
        nc.sync.dma_start(ltri, ltri_in[:, :])
        wr_sb = cpool.tile([128, DCH, E], FP32, tag="wr")
        nc.sync.dma_start(wr_sb, Wr_in[:, :, :])
        br_sb = cpool.tile([1, E], FP32, tag="br")
        nc.sync.dma_start(br_sb, br_in[:, :])
        # token-id iota: tokid[p, m] = m*128 + p
        tokid_i = cpool.tile([128, M], I32, tag="tokid_i")
        nc.gpsimd.iota(tokid_i, pattern=[[128, M]], base=0,
                       channel_multiplier=1)
        tokid_f = cpool.tile([128, M], FP32, tag="tokid_f")
        nc.vector.tensor_copy(tokid_f, tokid_i)

        # ---------- P1: router (own shard, fp32) ----------
        with tc.tile_pool(name="psr", bufs=2, space="PSUM") as psr:
            E_sb = xp.tile([128, TT, E], FP32, tag="E_sb")
            M8 = xp.tile([128, TT, 8], FP32, tag="M8")
            I8 = xp.tile([128, TT, 8], U16, tag="I8")
            Z_sb = xp.tile([128, TT], FP32, tag="Z_sb")
            RT_loc = xp.tile([128, TT, 4], FP32, tag="RT_loc")

            TQT = TT // 4  # t-tiles per xsb quarter
            for t in range(TT):
                if t % TQT == 0 and t // TQT + 2 < 4:
                    load_xq(t // TQT + 2)  # prefetch next-next quarter
                ps = psr.tile([128, E], FP32, tag="psr")
                xq = xsbq[t // TQT]
                tl = t % TQT
                for dch in range(DCH):
                    nc.tensor.matmul(
                        ps, lhsT=xq[:, dch, tl * 128:(tl + 1) * 128],
                        rhs=wr_sb[:, dch, :],
                        start=(dch == 0), stop=False)
                nc.tensor.matmul(ps, lhsT=ones1f, rhs=br_sb[:, :],
                                 start=False, stop=True)
                nc.scalar.activation(E_sb[:, t, :], ps, AF.Exp)
                nc.vector.max(M8[:, t, :], E_sb[:, t, :])
                nc.vector.max_index(I8[:, t, :], M8[:, t, :], E_sb[:, t, :])
            nc.vector.tensor_reduce(Z_sb, E_sb, AX.X, ALU.add)
            rZ = xp.tile([128, TT], FP32, tag="rZ")
            nc.vector.reciprocal(rZ, Z_sb)
            nc.vector.tensor_copy(RT_loc[:, :, 0], I8[:, :, 0])
            nc.vector.tensor_tensor(RT_loc[:, :, 1], M8[:, :, 0], rZ,
                                    ALU.mult)
            nc.vector.tensor_copy(RT_loc[:, :, 2], I8[:, :, 1])
            nc.vector.tensor_tensor(RT_loc[:, :, 3], M8[:, :, 1], rZ,
                                    ALU.mult)
            nc.sync.dma_start(routing_local,
                              RT_loc.rearrange("p t q -> p (t q)"))
            if dbg:
                nc.sync.dma_start(dbg_rt[:, :],
                                  RT_loc.rearrange("p t q -> p (t q)"))

        # zero the dispatch tables (padded slots must read token 0 / gate 0)
        zbig = xp.tile([128, 1344], FP32, tag="zbig")
        nc.vector.memset(zbig, 0.0)
        nc.sync.dma_start(disp_rec.rearrange("(p w) c -> p (w c)", p=128),
                          zbig)
        nc.sync.dma_start(disp_recB.rearrange("(p w) c -> p (w c)", p=128),
                          zbig)

        nc.gpsimd.collective_compute(
            "AllGather", ALU.bypass,
            replica_groups=[list(range(NC))],
            ins=[routing_local.opt()], outs=[routing_all.opt()])
        xp_cm.__exit__(None, None, None)  # free xsb

        # W2 load starts once xsb's SBUF is free
        wts2_cm = tc.tile_pool(name="wts2", bufs=1)
        wts2 = wts2_cm.__enter__()
        W2s = wts2.tile([128, HCH, D], BF16, tag="W2s")
        nc.sync.dma_start(W2s, W2_in[:, :, :])
        # b2 broadcast to all 128 partitions via rank-1 matmul
        b2bc = wts1.tile([128, D], FP32, tag="b2bc")

        selctx = [tc.tile_pool(name="sel", bufs=1),
                  tc.tile_pool(name="selsm", bufs=1),
                  tc.tile_pool(name="pscnt", bufs=2, space="PSUM")]
        sel, ssm, pscnt = [c.__enter__() for c in selctx]
        if True:
            for dh in range(NDH):
                pb = pscnt.tile([128, DHN], FP32, tag="pscnt")
                nc.tensor.matmul(pb, lhsT=ones1b,
                                 rhs=b2s[:, dh * DHN:(dh + 1) * DHN],
                                 start=True, stop=True)
                nc.vector.tensor_copy(b2bc[:, dh * DHN:(dh + 1) * DHN], pb)

            # ---------- P2: per-expert (sharded) selection ----------
            # RTA[p, m, q], token n = m*128+p, q = (i1, g1, i2, g2)
            RTA = sel.tile([128, M, 4], FP32, tag="RTA")
            nc.sync.dma_start(
                RTA, routing_all.rearrange("r p (t q) -> p r t q", q=4))
            # FFN weight loads: issued after RTA in program order so the
            # selection phase is not stuck behind them on the DMA queue
            nc.sync.dma_start(W1s, W1_in[:, :, :])
            nc.sync.dma_start(b1s, b1_in[:, :])
            i1f = RTA[:, :, 0]
            g1f = RTA[:, :, 1]
            i2f = RTA[:, :, 2]
            g2f = RTA[:, :, 3]

            # A_own[token] = gate if token routed to OUR expert, else 0
            A_own = sel.tile([128, M], FP32, tag="A_own")
            tmpM = sel.tile([128, M], FP32, tag="tmpM")
            nc.vector.scalar_tensor_tensor(
                A_own, i1f, rankf, g1f, op0=ALU.is_equal, op1=ALU.mult)
            nc.vector.scalar_tensor_tensor(
                tmpM, i2f, rankf, g2f, op0=ALU.is_equal, op1=ALU.mult)
            nc.vector.tensor_tensor(A_own, A_own, tmpM, ALU.add)

            big = sel.tile([128, M], FP32, tag="big")
            cnt1 = sel.tile([128, 1], FP32, tag="cnt1")
            Ktgt = sel.tile([128, 1], FP32, tag="Ktgt")
            lo = sel.tile([128, 1], I32, tag="lo")
            hi = sel.tile([128, 1], I32, tag="hi")
            mid = sel.tile([128, 1], I32, tag="mid")
            condi = sel.tile([128, 1], I32, tag="condi")

            nc.vector.tensor_scalar(big, A_own, 0.0, None, op0=ALU.is_gt)
            nc.vector.tensor_reduce(cnt1, big, AX.X, ALU.add)
            pc0 = pscnt.tile([128, 1], FP32, tag="pscnt")
            nc.tensor.matmul(pc0, lhsT=ones128f, rhs=cnt1, start=True,
                             stop=True)
            nc.vector.tensor_scalar(Ktgt, pc0, float(CAP), None, op0=ALU.min)

            nc.vector.memset(lo, 0)
            nc.vector.memset(hi, 0x3F800000)
            for it in range(cfg.NBIS):
                nc.vector.tensor_tensor(mid, lo, hi, ALU.add)
                nc.vector.tensor_tensor(mid, mid, one_i,
                                        ALU.logical_shift_right)
                midf = mid.bitcast(FP32)
                nc.vector.tensor_scalar(big, A_own, midf, None,
                                        op0=ALU.is_gt)
                nc.vector.tensor_reduce(cnt1, big, AX.X, ALU.add)
                pc = pscnt.tile([128, 1], FP32, tag="pscnt")
                nc.tensor.matmul(pc, lhsT=ones128f, rhs=cnt1, start=True,
                                 stop=True)
                nc.vector.tensor_tensor(condi, pc, Ktgt, ALU.is_ge)
                nc.vector.copy_predicated(lo, condi, mid)
                nc.vector.tensor_tensor(condi, pc, Ktgt, ALU.is_lt)
                nc.vector.copy_predicated(hi, condi, mid)

            thrf = lo.bitcast(FP32)
            if dbg:
                nc.sync.dma_start(dbg_thr[:, :], lo)
            keepf = sel.tile([128, M], FP32, tag="keepf")
            nc.vector.tensor_scalar(keepf, A_own, thrf, None, op0=ALU.is_gt)
            keepi = sel.tile([128, M], I32, tag="keepi")
            nc.vector.tensor_copy(keepi, keepf)

            # positions within our expert buffer: slot order = p*M + m
            rp = sel.tile([128, M], FP32, tag="rp")
            nc.vector.tensor_tensor_scan(
                rp, keepf, zerosM, initial=0.0, op0=ALU.add, op1=ALU.add)
            totb = sel.tile([128, 1], BF16, tag="totb")
            nc.vector.tensor_copy(totb, rp[:, M - 1:M])
            pe_x = pscnt.tile([128, 1], FP32, tag="pscnt")
            nc.tensor.matmul(pe_x, lhsT=ltri, rhs=totb, start=True, stop=True)
            excl = sel.tile([128, 1], FP32, tag="excl")
            nc.vector.tensor_copy(excl, pe_x)
            pos = sel.tile([128, M], FP32, tag="pos")
            nc.vector.scalar_tensor_tensor(
                pos, rp, excl, keepf, op0=ALU.add, op1=ALU.subtract)

            if dbg:
                nc.sync.dma_start(dbg_pos[:, :], pos)
                nc.sync.dma_start(dbg_keep[:, :], keepf)
            # share (pos or BIG) with every core for the combine phase
            posx = sel.tile([128, M], FP32, tag="posx")
            nc.vector.tensor_copy(posx, bigP)
            nc.vector.copy_predicated(posx, keepi, pos)
            nc.sync.dma_start(posx_local, posx)
            nc.gpsimd.collective_compute(
                "AllGather", ALU.bypass,
                replica_groups=[list(range(NC))],
                ins=[posx_local.opt()], outs=[posx_all.opt()])

            # dispatch scatter indices: with per-expert sharding a token
            # reaches our expert via at most one of its top-2 choices, so
            # one record per token suffices (idx = kept ? pos : dump,
            # gate = A_own)
            sci16 = sel.tile([128, M], I16, tag="sci16")
            offd = sel.tile([128, M], FP32, tag="offd")
            nc.vector.tensor_copy(offd, bigT)
            nc.vector.copy_predicated(offd, keepi, pos)
            nc.vector.tensor_copy(sci16, offd)
            if dbg:
                nc.sync.dma_start(dbg_sci[:, :], sci16)
            nc.sync.dma_start(scat_bounce, sci16)

            # combine flat idx for own tokens, from the AllGathered
            # per-expert positions: raw = pos[e_k] + e_k*CAPP, dropped->BIG
            PXA = sel.tile([128, E, TT], FP32, tag="PXA")
            nc.sync.dma_start(
                PXA,
                posx_all.rearrange("r p m -> p r m")[
                    :, :, bass.ds(rank_sp * TT, TT)])
            nc.vector.tensor_tensor(
                PXA, PXA, ebaseE.unsqueeze(2).broadcast_to((128, E, TT)),
                ALU.add)
            ci16 = sel.tile([128, K, TT], I16, tag="ci16")
            rawk = sel.tile([128, TT], FP32, tag="rawk")
            tmpT = sel.tile([128, TT], FP32, tag="tmpT")
            keepT = sel.tile([128, TT], I32, tag="keepT")
            fck = sel.tile([128, TT], FP32, tag="fck")
            own0 = bass.ds(rank_sp * TT, TT)
            for k in range(K):
                ikf_o = (i1f if k == 0 else i2f)[:, own0]
                first = True
                for e in range(E):
                    dst = rawk if first else tmpT
                    nc.vector.scalar_tensor_tensor(
                        dst, ikf_o, float(e), PXA[:, e, :],
                        op0=ALU.is_equal, op1=ALU.mult)
                    if not first:
                        nc.vector.tensor_tensor(rawk, rawk, tmpT, ALU.add)
                    first = False
                # kept iff raw's pos-part < NSLOT (dropped marker >= NSLOT)
                nc.vector.tensor_scalar(tmpT, rawk, float(NSLOT), None,
                                        op0=ALU.is_lt)
                nc.vector.tensor_copy(keepT, tmpT)
                nc.vector.tensor_copy(fck, zslotc)
                nc.vector.copy_predicated(fck, keepT, rawk)
                nc.vector.tensor_copy(ci16[:, k, :], fck)
            nc.sync.dma_start(flat_bounce, ci16)
            if dbg:
                nc.sync.dma_start(dbg_ci[:, :],
                                  ci16.rearrange("p k t -> p (k t)"))
                nc.sync.dma_start(dbg_pxa[:, :],
                                  PXA.rearrange("p e t -> p (e t)"))
            # scatter (tok, gate) records into disp_rec rows (8B payload,
            # 256B row stride)
            pay = sel.tile([128, M, 2], FP32, tag="pay")
            sidx = sel.tile([128, M * 8], I16, tag="sidx")
            nc.vector.tensor_copy(pay[:, :, 0], tokid_f)
            nc.vector.tensor_copy(pay[:, :, 1], A_own)
            # idx item i = m*128+p -> [p%16, m*8+p//16]; fold once into
            # partitions 0:16, then replicate via cheap SBUF copies
            nc.sync.dma_start(
                sidx[0:16, :].rearrange("p (m h) -> p m h", h=8),
                scat_bounce.rearrange("(ph pl) m -> pl m ph", pl=16))
            for g in range(1, 8):
                nc.sync.dma_start(sidx[g * 16:(g + 1) * 16, :],
                                  sidx[0:16, :])
            SC = min(8, M)  # m-groups per scatter (<=1024 items)
            # alternate between two tables to break the per-call WAW chain;
            # each slot is written by exactly one call, so table_A + table_B
            # reconstructs the records
            for ci, m0 in enumerate(range(0, M, SC)):
                nc.gpsimd.dma_scatter_add(
                    out_ap=(disp_rec if ci % 2 == 0 else disp_recB)[:, 0:2],
                    in_ap=pay[:, m0:m0 + SC, :],
                    idxs_ap=sidx[:, m0 * 8:(m0 + SC) * 8],
                    num_idxs=SC * 128,
                    num_idxs_reg=SC * 128,
                    elem_size=2,
                    elem_step=64)

            # combine gather idx tile [16, K*TT*8] -> [128, K*TT*8] padded
            cidx = keepp.tile([128, K * TT * 8], I16, tag="cidx")
            nc.sync.dma_start(
                cidx[0:16, :],
                flat_bounce.rearrange("(ph pl) k t -> pl k t ph", pl=16))
            for g in range(1, 8):
                nc.sync.dma_start(cidx[g * 16:(g + 1) * 16, :], cidx[0:16, :])

            # dispatch idx tile + slot gates (sum of the two tables)
            gdisp = keepp.tile([128, CAPP // 128], FP32, tag="gdisp")
            gdispB = sel.tile([128, CAPP // 128], FP32, tag="gdispB")
            nc.sync.dma_start(
                gdisp,
                disp_rec[0:CAPP, :].rearrange(
                    "(c s) w -> s c w", s=128)[:, :, 1])
            nc.sync.dma_start(
                gdispB,
                disp_recB[0:CAPP, :].rearrange(
                    "(c s) w -> s c w", s=128)[:, :, 1])
            nc.vector.tensor_tensor(gdisp, gdisp, gdispB, ALU.add)
            if dbg:
                nc.sync.dma_start(dbg_gd[:, :], gdisp)
            tokf16 = sel.tile([16, CAPP // 16], FP32, tag="tokf16")
            tokf16B = sel.tile([16, CAPP // 16], FP32, tag="tokf16B")
            nc.sync.dma_start(
                tokf16,
                disp_rec[0:CAPP, :].rearrange(
                    "(c q) w -> q c w", q=16)[:, :, 0])
            nc.sync.dma_start(
                tokf16B,
                disp_recB[0:CAPP, :].rearrange(
                    "(c q) w -> q c w", q=16)[:, :, 0])
            nc.vector.tensor_tensor(tokf16, tokf16, tokf16B, ALU.add)
            if dbg:
                nc.sync.dma_start(dbg_tok[:, :], tokf16)
            dIdx = keepp.tile([128, CAPP // 16], I16, tag="dIdx")
            nc.vector.tensor_copy(dIdx[0:16, :], tokf16)
            for g in range(1, 8):
                nc.sync.dma_start(dIdx[g * 16:(g + 1) * 16, :], dIdx[0:16, :])

        # ---------- P3: expert FFN ----------
        for c_ in reversed(selctx):
            c_.__exit__(None, None, None)
        with tc.tile_pool(name="ffn", bufs=2) as ffn, \
             tc.tile_pool(name="ht", bufs=1) as htp, \
             tc.tile_pool(name="ps1", bufs=2, space="PSUM") as ps1p, \
             tc.tile_pool(name="ps2", bufs=2, space="PSUM") as ps2p:

            for c in range(NCHUNK):
                xTg = ffn.tile([128, DCH, CCHUNK], BF16, tag="xTg")
                nc.gpsimd.dma_gather(
                    out_ap=xTg,
                    in_ap=x_bf16[:, :],
                    idxs_ap=dIdx[:, c * (CCHUNK // 16):
                                 (c + 1) * (CCHUNK // 16)],
                    num_idxs=CCHUNK,
                    num_idxs_reg=CCHUNK,
                    elem_size=D,
                    transpose=True)
                if dbg and c == 0:
                    nc.sync.dma_start(
                        dbg_xtg[:, :], xTg.rearrange("p a b -> p (a b)"))
                hT = htp.tile([128, HCH, CCHUNK], BF16, tag="hT")
                for j in range(HCH):
                    ps1 = ps1p.tile([128, CCHUNK], FP32, tag="ps1")
                    for dch in range(DCH):
                        nc.tensor.matmul(
                            ps1, lhsT=W1s[:, dch, j * 128:(j + 1) * 128],
                            rhs=xTg[:, dch, :],
                            start=(dch == 0), stop=(dch == DCH - 1))
                    sgt = ffn.tile([128, CCHUNK], FP32, tag="sgt")
                    nc.scalar.activation(sgt, ps1, AF.Sigmoid,
                                         bias=b1s[:, j:j + 1])
                    nc.vector.scalar_tensor_tensor(
                        hT[:, j, :], ps1, b1s[:, j:j + 1], sgt,
                        op0=ALU.add, op1=ALU.mult)
                for cs in range(CCHUNK // 128):
                    col = c * (CCHUNK // 128) + cs
                    # b2g = b2 * gate (per-token row scalar)
                    b2g = ffn.tile([128, D], FP32, tag="b2g")
                    nc.vector.tensor_scalar(
                        b2g, b2bc, gdisp[:, col:col + 1], None, op0=ALU.mult)
                    osb = ffn.tile([128, D], BF16, tag="osb")
                    for dh in range(NDH):
                        ps2 = ps2p.tile([128, DHN], FP32, tag="ps2")
                        for j in range(HCH):
                            nc.tensor.matmul(
                                ps2,
                                lhsT=hT[:, j, cs * 128:(cs + 1) * 128],
                                rhs=W2s[:, j, dh * DHN:(dh + 1) * DHN],
                                start=(j == 0), stop=(j == HCH - 1))
                        # osb = ps2*gate + b2*gate
                        nc.vector.scalar_tensor_tensor(
                            osb[:, dh * DHN:(dh + 1) * DHN], ps2,
                            gdisp[:, col:col + 1],
                            b2g[:, dh * DHN:(dh + 1) * DHN],
                            op0=ALU.mult, op1=ALU.add)
                    nc.sync.dma_start(
                        out_e[col * 128:(col + 1) * 128, :], osb)

        nc.gpsimd.collective_compute(
            "AllGather", ALU.bypass,
            replica_groups=[list(range(NC))],
            ins=[out_e.opt()], outs=[all_out.opt()])

        wts2_cm.__exit__(None, None, None)
        wts1_cm.__exit__(None, None, None)

        # ---------- P4: combine own shard ----------
        with tc.tile_pool(name="comb", bufs=2) as comb:
            GC = min(8, TT)  # t-tiles per gather (<=1024 idxs)
            gk_tiles = []
            for k in range(K):
                gk = comb.tile([128, TT, D], BF16, tag=f"gk{k}")
                gk_tiles.append(gk)
            for t0 in range(0, TT, GC):
                for k in range(K):
                    nc.gpsimd.dma_gather(
                        out_ap=gk_tiles[k][:, t0:t0 + GC, :],
                        idxs_ap=cidx[:, k * TT * 8 + t0 * 8:
                                     k * TT * 8 + (t0 + GC) * 8],
                        in_ap=all_out.rearrange("r c d -> (r c) d"),
                        num_idxs=GC * 128,
                        num_idxs_reg=GC * 128,
                        elem_size=D,
                        transpose=False)
                for t in range(t0, t0 + GC):
                    ysb = comb.tile([128, D], FP32, tag="ysb")
                    nc.vector.tensor_tensor(ysb, gk_tiles[0][:, t, :],
                                            gk_tiles[1][:, t, :], ALU.add)
                    nc.sync.dma_start(y_out[t * 128:(t + 1) * 128, :], ysb)

        cpool_cm.__exit__(None, None, None)
        keepp_cm.__exit__(None, None, None)
        dramp_cm.__exit__(None, None, None)

    nc.compile()
    return nc


# ---------------- host-side staging ----------------

def stage_inputs(cfg: Cfg, x, Wr, br, W1, b1, W2, b2):
    """x: [N, D] fp32; returns list of per-core input dicts."""
    E, D, H, TPC, NC = cfg.E, cfg.D, cfg.H, cfg.TPC, cfg.ncores
    DCH, HCH = cfg.DCH, cfg.HCH
    x = np.ascontiguousarray(x, np.float32)
    x_bf = x.astype(bfloat16_np())
    ltri = np.tril(np.ones((128, 128), np.float32), -1).astype(bfloat16_np())
    in_maps = []
    for r in range(NC):
        shard = x[r * TPC:(r + 1) * TPC]
        m = {
            "xT_shard": np.ascontiguousarray(shard.T),
            "x_bf16": x_bf,
            "rank_in": np.full((128, 1), r, np.float32),
            "Wr_in": np.ascontiguousarray(
                Wr.reshape(DCH, 128, E).transpose(1, 0, 2)).astype(np.float32),
            "br_in": br.reshape(1, E).astype(np.float32),
            "W1_in": np.ascontiguousarray(
                W1[r].reshape(DCH, 128, H).transpose(1, 0, 2)
            ).astype(bfloat16_np()),
            "W2_in": np.ascontiguousarray(
                W2[r].reshape(HCH, 128, D).transpose(1, 0, 2)
            ).astype(bfloat16_np()),
            "b1_in": np.ascontiguousarray(
                b1[r].reshape(HCH, 128).T).astype(np.float32),
            "b2_in": b2[r].reshape(1, D).astype(np.float32).astype(
                bfloat16_np()),
            "ltri_in": ltri,
        }
        in_maps.append(m)
    return in_maps


def bfloat16_np():
    import ml_dtypes
    return ml_dtypes.bfloat16


def unshard(cfg: Cfg, results, B, T):
    ys = [results[r]["y_out"] for r in range(cfg.ncores)]
    y = np.concatenate(ys, axis=0)
    return y.reshape(B, T, cfg.D)


# ---------------- problem binding ----------------

import math as _math

B, T = 8, 2048
_N = B * T
_D = 1024
_CAP = int(_math.ceil(1.2 * _N / 8))  # 2458

_CACHE = {}


def _get_nc():
    if "nc" not in _CACHE:
        cfg = Cfg(D=_D, H=4096, TPC=_N // 8, cap=_CAP, CAPP=2560, CCHUNK=512)
        _CACHE["cfg"] = cfg
        _CACHE["nc"] = build(cfg)
    return _CACHE["cfg"], _CACHE["nc"]


def kernel(x_btd, Wr, br, W1, b1, W2, b2):
    from concourse.bass_utils import run_bass_kernel_spmd

    cfg, nc = _get_nc()
    x = np.ascontiguousarray(np.asarray(x_btd), np.float32).reshape(_N, _D)
    in_maps = stage_inputs(
        cfg, x, np.asarray(Wr), np.asarray(br), np.asarray(W1),
        np.asarray(b1), np.asarray(W2), np.asarray(b2))
    res = run_bass_kernel_spmd(nc, in_maps, list(range(8)))
    ys = [res.results[r]["y_out"] for r in range(8)]
    y = np.concatenate(ys, axis=0).astype(np.float32)
    return y.reshape(B, T, _D)
